# revision 15
# baseline (speedup 1.0000x reference)
import sys

for _p in ('/opt/trn_rl_repo', '/root/.axon_site'):
    if _p not in sys.path:
        sys.path.insert(0, _p)

import numpy as np

B, H, W = 8, 512, 512
K = 3
NCORES = 8
# padded image: 1 zero row/col before, 2 zero rows/cols after (cols padded
# further so shifted views stay in range and rows stay 4B-aligned)
HP, WP = H + 3, W + 8
NBLK = 4          # row blocks of 128 partitions packed along the free dim
AW = 520          # A tile width (Ipad cols 0..519)
DW = 516          # Dx/Dy/Dxy tile width

_compiled = None


def _build():
    import concourse.bacc as bacc
    import concourse.mybir as mybir
    from concourse.tile import TileContext

    f32, f16 = mybir.dt.float32, mybir.dt.float16
    ALU = mybir.AluOpType
    ACTF = mybir.ActivationFunctionType

    nc = bacc.Bacc("TRN2", target_bir_lowering=False, debug=False,
                   num_devices=NCORES)
    ipad = nc.dram_tensor("ipad", [HP, WP], f16, kind="ExternalInput")
    # offsets pre-cast to fp16 and pre-arranged on host: plane 2k = ly_k,
    # plane 2k+1 = lx_k, each already in [p, j, c] partition-major layout
    off = nc.dram_tensor("off", [2 * K * K, 128, NBLK, W], f16,
                         kind="ExternalInput")
    # stack of diag(w_k) matrices used as PE stationary weights
    wdg = nc.dram_tensor("wdg", [128, K * K, 128], f16, kind="ExternalInput")
    out = nc.dram_tensor("out", [H, W], f16, kind="ExternalOutput")

    with TileContext(nc) as tc:
        with (
            tc.tile_pool(name="img", bufs=1) as ip,
            tc.tile_pool(name="l16", bufs=18) as lp,
            tc.tile_pool(name="tmp", bufs=4) as tp,
            tc.tile_pool(name="cst", bufs=1) as cp,
            tc.tile_pool(name="psum", bufs=1, space="PSUM") as pp,
        ):
            psum = pp.tile([128, NBLK, W], f32, name="psum")

            # ---- DMA: two HWDGE rings (SP + ACT). GpSimd/Pool stays
            # completely idle: any concurrent Pool (or ACT compute) activity
            # contends DVE's SBUF ports and slows tensor_tensor ~4x.
            A = {}

            def load_img(dy, eng):
                A[dy] = ip.tile([128, NBLK, AW], f16, tag=f"A{dy}",
                                name=f"A{dy}")
                eng.dma_start(
                    out=A[dy][:],
                    in_=ipad[dy + 1:dy + 513, 0:AW].rearrange(
                        "(j p) c -> p j c", p=128))

            lys, lxs = {}, {}

            def load_lylx(k, eng):
                lxs[k] = lp.tile([128, NBLK, W], f16, tag="l", name=f"lx{k}")
                eng.dma_start(out=lxs[k][:], in_=off[2 * k + 1])
                lys[k] = lp.tile([128, NBLK, W], f16, tag="l", name=f"ly{k}")
                eng.dma_start(out=lys[k][:], in_=off[2 * k])

            # need-ordered loads; wd and A(2) are pushed past the critical
            # early window with explicit scheduler wait hints so their
            # transfers can't jump ahead of the k1..k3 offset loads.
            # A(-1) and A(0) are each split across BOTH rings so the first
            # diffs can start ~1.5us sooner.
            def load_img_xring(dy, eng_lo, eng_hi):
                A[dy] = ip.tile([128, NBLK, AW], f16, tag=f"A{dy}",
                                name=f"A{dy}")
                for h, eng in ((0, eng_lo), (1, eng_hi)):
                    eng.dma_start(
                        out=A[dy][:, 2 * h:2 * h + 2, :],
                        in_=ipad[dy + 1 + 256 * h:dy + 257 + 256 * h, 0:AW]
                        .rearrange("(j p) c -> p j c", p=128))

            load_img_xring(-1, nc.sync, nc.scalar)
            load_img_xring(0, nc.scalar, nc.sync)
            load_lylx(0, nc.sync)
            load_lylx(1, nc.scalar)
            load_img(1, nc.sync)
            load_lylx(2, nc.sync)
            load_lylx(3, nc.scalar)
            with tc.tile_wait_until(0.010):
                wd = cp.tile([128, K * K, 128], f16, tag="wd", name="wd")
                nc.scalar.dma_start(out=wd[:], in_=wdg[:])
            load_lylx(4, nc.sync)
            load_lylx(5, nc.scalar)
            load_lylx(6, nc.sync)
            with tc.tile_wait_until(0.018):
                load_img(2, nc.scalar)
            load_lylx(7, nc.scalar)
            load_lylx(8, nc.sync)

            # ---- diffs of the padded image (all DVE)
            Dx, Dy, Dxy = {}, {}, {}

            def make_dx(dy):
                Dx[dy] = ip.tile([128, NBLK, DW], f16, tag=f"D{dy}",
                                 name=f"D{dy}")
                nc.vector.tensor_tensor(Dx[dy][:], A[dy][:, :, 1:1 + DW],
                                        A[dy][:, :, 0:DW], ALU.subtract)

            def make_dy(j):
                Dy[j] = ip.tile([128, NBLK, DW], f16, tag=f"Y{j}",
                                name=f"Y{j}")
                nc.vector.tensor_tensor(Dy[j][:], A[j + 1][:, :, 0:DW],
                                        A[j][:, :, 0:DW], ALU.subtract)

            def make_dxy(j):
                Dxy[j] = ip.tile([128, NBLK, DW], f16, tag=f"X{j}",
                                 name=f"X{j}")
                nc.vector.tensor_tensor(Dxy[j][:], Dx[j + 1][:],
                                        Dx[j][:], ALU.subtract)

            def iview(dy, q):
                return A[dy][:, :, q:q + W]

            def mm_i0(k, start):
                ky, q = k // K - 1, k % K
                wk = wd[:, k, :]
                for j in range(NBLK):
                    nc.tensor.matmul(psum[:, j, :], wk, iview(ky, q)[:, j, :],
                                     start=start, stop=False)

            # per tap: v*w_k = w_k*I0 + w_k*t + w_k*t2
            #   t  = lx*Dx[ky]
            #   t2 = ly*(Dy[ky] + lx*Dxy[ky])
            tap_t = {}
            tap_t2 = {}

            def tap_products(k, split_last=False):
                ky, q = k // K - 1, k % K
                ly, lx = lys[k], lxs[k]
                t = tp.tile([128, NBLK, W], f16, tag="t", bufs=6,
                            name=f"t{k}")
                t3 = tp.tile([128, NBLK, W], f16, tag="t3", name=f"t3{k}")
                t2 = tp.tile([128, NBLK, W], f16, tag="t2", bufs=6,
                             name=f"t2{k}")
                nc.vector.tensor_tensor(t[:], lx[:], Dx[ky][:, :, q:q + W],
                                        ALU.mult)
                nc.vector.tensor_tensor(t3[:], lx[:], Dxy[ky][:, :, q:q + W],
                                        ALU.mult)
                nc.vector.tensor_tensor(t2[:], t3[:], Dy[ky][:, :, q:q + W],
                                        ALU.add)
                if split_last:
                    for j in range(NBLK):
                        nc.vector.tensor_tensor(t2[:, j, :], ly[:, j, :],
                                                t2[:, j, :], ALU.mult)
                else:
                    nc.vector.tensor_tensor(t2[:], ly[:], t2[:], ALU.mult)
                tap_t[k] = t
                tap_t2[k] = t2

            def mm_products(k, stop=False):
                # t2 matmuls first: frees the t2 pool slot sooner (its DVE
                # producer chain is the longest); interleave per-bank when
                # stopping so banks close (and drain) one by one
                wk = wd[:, k, :]
                if stop:
                    for j in range(NBLK):
                        nc.tensor.matmul(psum[:, j, :], wk,
                                         tap_t[k][:, j, :],
                                         start=False, stop=False)
                        nc.tensor.matmul(psum[:, j, :], wk,
                                         tap_t2[k][:, j, :],
                                         start=False, stop=True)
                    return
                for j in range(NBLK):
                    nc.tensor.matmul(psum[:, j, :], wk, tap_t2[k][:, j, :],
                                     start=False, stop=False)
                for j in range(NBLK):
                    nc.tensor.matmul(psum[:, j, :], wk, tap_t[k][:, j, :],
                                     start=False, stop=False)

            # ---- DVE stream: just-in-time diffs, taps k0..k8; I0 matmuls
            # are interleaved per-tap so the wd load isn't pulled early
            make_dx(-1)
            make_dx(0)
            make_dy(-1)
            make_dxy(-1)
            for k in (0, 1, 2):
                tap_products(k)
                mm_i0(k, start=(k == 0))
                mm_products(k)
            make_dx(1)
            make_dy(0)
            make_dxy(0)
            for k in (3, 4, 5):
                tap_products(k)
                mm_i0(k, start=False)
                mm_products(k)
            make_dx(2)
            make_dy(1)
            make_dxy(1)
            # I0 matmuls of the late taps go here: they only need A + wd,
            # and emitting them before the last products keeps them out of
            # the drain tail
            mm_i0(6, start=False)
            mm_i0(7, start=False)
            mm_i0(8, start=False)
            for k in (6, 7):
                tap_products(k)
                mm_products(k)
            tap_products(8, split_last=True)
            mm_products(8, stop=True)

            # ---- per-bank copy + store; copies split ACT/DVE (both idle by
            # now) with each engine's copies consecutive on its queue, and
            # stores split across the two HWDGE rings in bank order
            res = cp.tile([128, NBLK, W], f16, tag="res", name="res")
            outv = out.rearrange("(j p) c -> p j c", p=128)
            nc.scalar.activation(res[:, 0, :], psum[:, 0, :], ACTF.Copy)
            nc.sync.dma_start(out=outv[:, 0, :], in_=res[:, 0, :])
            nc.vector.tensor_copy(res[:, 1, :], psum[:, 1, :])
            nc.scalar.dma_start(out=outv[:, 1, :], in_=res[:, 1, :])
            nc.scalar.activation(res[:, 2, :], psum[:, 2, :], ACTF.Copy)
            nc.sync.dma_start(out=outv[:, 2, :], in_=res[:, 2, :])
            nc.vector.tensor_copy(res[:, 3, :], psum[:, 3, :])
            nc.scalar.dma_start(out=outv[:, 3, :], in_=res[:, 3, :])

    nc.compile()
    return nc


def kernel(input, weight, offset):
    global _compiled
    from concourse.bass_utils import run_bass_kernel_spmd

    if _compiled is None:
        _compiled = _build()
    nc = _compiled

    input = np.asarray(input, dtype=np.float32)
    offset = np.asarray(offset, dtype=np.float32)
    w9 = np.asarray(weight, dtype=np.float32).reshape(K * K)
    wdg = np.zeros((128, K * K, 128), np.float16)
    idx = np.arange(128)
    for k in range(K * K):
        wdg[idx, k, idx] = w9[k].astype(np.float16)

    ipad = np.zeros((B, HP, WP), np.float16)
    ipad[:, 1:H + 1, 1:W + 1] = input.astype(np.float16)

    # host-side cast + partition-major rearrange of the offsets:
    # (18, 512, 512) fp32 -> (18, 128, NBLK, 512) fp16 with row r = 128*j + p
    off16 = np.ascontiguousarray(
        offset.astype(np.float16)
        .reshape(B, 2 * K * K, NBLK, 128, W)
        .transpose(0, 1, 3, 2, 4))

    in_maps = [
        {"ipad": ipad[b], "off": off16[b], "wdg": wdg} for b in range(B)
    ]
    res = run_bass_kernel_spmd(nc, in_maps, list(range(NCORES)), trace=False)
    return np.stack(
        [res.results[b]["out"].astype(np.float32) for b in range(B)], axis=0)


# revision 17
# speedup vs baseline: 1.2014x; 1.2014x over previous
import sys

for _p in ('/opt/trn_rl_repo', '/root/.axon_site'):
    if _p not in sys.path:
        sys.path.insert(0, _p)

import numpy as np

B, H, W = 8, 512, 512
K = 3
NCORES = 8
# padded image: 1 zero row/col before, 2 zero rows/cols after (cols padded
# further so shifted views stay in range and rows stay 4B-aligned)
HP, WP = H + 3, W + 8
NBLK = 4          # row blocks of 128 partitions packed along the free dim
AW = 520          # A tile width (Ipad cols 0..519)
DW = 516          # Dx/Dy/Dxy tile width

_compiled = None


def _build():
    import concourse.bacc as bacc
    import concourse.mybir as mybir
    from concourse.tile import TileContext

    f32, f16 = mybir.dt.float32, mybir.dt.float16
    ALU = mybir.AluOpType
    ACTF = mybir.ActivationFunctionType

    nc = bacc.Bacc("TRN2", target_bir_lowering=False, debug=False,
                   num_devices=NCORES)
    ipad = nc.dram_tensor("ipad", [HP, WP], f16, kind="ExternalInput")
    # offsets pre-cast to fp16 and pre-arranged on host: plane 2k = ly_k,
    # plane 2k+1 = lx_k, each already in [p, j, c] partition-major layout
    off = nc.dram_tensor("off", [2 * K * K, 128, NBLK, W], f16,
                         kind="ExternalInput")
    # stack of diag(w_k) matrices used as PE stationary weights
    wdg = nc.dram_tensor("wdg", [128, K * K, 128], f16, kind="ExternalInput")
    out = nc.dram_tensor("out", [H, W], f16, kind="ExternalOutput")

    with TileContext(nc) as tc:
        with (
            tc.tile_pool(name="img", bufs=1) as ip,
            tc.tile_pool(name="l16", bufs=18) as lp,
            tc.tile_pool(name="tmp", bufs=4) as tp,
            tc.tile_pool(name="cst", bufs=1) as cp,
            tc.tile_pool(name="psum", bufs=1, space="PSUM") as pp,
        ):
            psum = pp.tile([128, NBLK, W], f32, name="psum")

            # ---- DMA: two HWDGE rings (SP + ACT). GpSimd/Pool stays
            # completely idle: any concurrent Pool (or ACT compute) activity
            # contends DVE's SBUF ports and slows tensor_tensor ~4x.
            A = {}

            def load_img(dy, eng):
                A[dy] = ip.tile([128, NBLK, AW], f16, tag=f"A{dy}",
                                name=f"A{dy}")
                eng.dma_start(
                    out=A[dy][:],
                    in_=ipad[dy + 1:dy + 513, 0:AW].rearrange(
                        "(j p) c -> p j c", p=128))

            lys, lxs = {}, {}

            def load_lylx(k, eng):
                lxs[k] = lp.tile([128, NBLK, W], f16, tag="l", name=f"lx{k}")
                eng.dma_start(out=lxs[k][:], in_=off[2 * k + 1])
                lys[k] = lp.tile([128, NBLK, W], f16, tag="l", name=f"ly{k}")
                eng.dma_start(out=lys[k][:], in_=off[2 * k])

            # need-ordered loads; wd and A(2) are pushed past the critical
            # early window with explicit scheduler wait hints so their
            # transfers can't jump ahead of the k1..k3 offset loads
            load_img(-1, nc.sync)
            load_img(0, nc.scalar)
            load_lylx(0, nc.sync)
            load_lylx(1, nc.scalar)
            load_img(1, nc.sync)
            load_lylx(2, nc.sync)
            load_lylx(3, nc.scalar)
            with tc.tile_wait_until(0.010):
                wd = cp.tile([128, K * K, 128], f16, tag="wd", name="wd")
                nc.scalar.dma_start(out=wd[:], in_=wdg[:])
            load_lylx(4, nc.sync)
            load_lylx(5, nc.scalar)
            load_lylx(6, nc.sync)
            with tc.tile_wait_until(0.018):
                load_img(2, nc.scalar)
            load_lylx(7, nc.scalar)
            load_lylx(8, nc.sync)

            # ---- diffs of the padded image (all DVE)
            Dx, Dy, Dxy = {}, {}, {}

            def make_dx(dy):
                Dx[dy] = ip.tile([128, NBLK, DW], f16, tag=f"D{dy}",
                                 name=f"D{dy}")
                nc.vector.tensor_tensor(Dx[dy][:], A[dy][:, :, 1:1 + DW],
                                        A[dy][:, :, 0:DW], ALU.subtract)

            def make_dy(j):
                Dy[j] = ip.tile([128, NBLK, DW], f16, tag=f"Y{j}",
                                name=f"Y{j}")
                nc.vector.tensor_tensor(Dy[j][:], A[j + 1][:, :, 0:DW],
                                        A[j][:, :, 0:DW], ALU.subtract)

            def make_dxy(j):
                Dxy[j] = ip.tile([128, NBLK, DW], f16, tag=f"X{j}",
                                 name=f"X{j}")
                nc.vector.tensor_tensor(Dxy[j][:], Dx[j + 1][:],
                                        Dx[j][:], ALU.subtract)

            def iview(dy, q):
                return A[dy][:, :, q:q + W]

            def mm_i0(k, start):
                ky, q = k // K - 1, k % K
                wk = wd[:, k, :]
                for j in range(NBLK):
                    nc.tensor.matmul(psum[:, j, :], wk, iview(ky, q)[:, j, :],
                                     start=start, stop=False)

            # per tap: v*w_k = w_k*I0 + w_k*t + w_k*t2
            #   t  = lx*Dx[ky]
            #   t2 = ly*(Dy[ky] + lx*Dxy[ky])
            tap_t = {}
            tap_t2 = {}

            def tap_products(k, split_last=False):
                ky, q = k // K - 1, k % K
                ly, lx = lys[k], lxs[k]
                t = tp.tile([128, NBLK, W], f16, tag="t", bufs=6,
                            name=f"t{k}")
                t3 = tp.tile([128, NBLK, W], f16, tag="t3", name=f"t3{k}")
                t2 = tp.tile([128, NBLK, W], f16, tag="t2", bufs=6,
                             name=f"t2{k}")
                nc.vector.tensor_tensor(t[:], lx[:], Dx[ky][:, :, q:q + W],
                                        ALU.mult)
                nc.vector.tensor_tensor(t3[:], lx[:], Dxy[ky][:, :, q:q + W],
                                        ALU.mult)
                nc.vector.tensor_tensor(t2[:], t3[:], Dy[ky][:, :, q:q + W],
                                        ALU.add)
                if split_last:
                    for j in range(NBLK):
                        nc.vector.tensor_tensor(t2[:, j, :], ly[:, j, :],
                                                t2[:, j, :], ALU.mult)
                else:
                    nc.vector.tensor_tensor(t2[:], ly[:], t2[:], ALU.mult)
                tap_t[k] = t
                tap_t2[k] = t2

            def mm_products(k, stop=False):
                # t2 matmuls first: frees the t2 pool slot sooner (its DVE
                # producer chain is the longest); interleave per-bank when
                # stopping so banks close (and drain) one by one
                wk = wd[:, k, :]
                if stop:
                    for j in range(NBLK):
                        nc.tensor.matmul(psum[:, j, :], wk,
                                         tap_t[k][:, j, :],
                                         start=False, stop=False)
                        nc.tensor.matmul(psum[:, j, :], wk,
                                         tap_t2[k][:, j, :],
                                         start=False, stop=True)
                    return
                for j in range(NBLK):
                    nc.tensor.matmul(psum[:, j, :], wk, tap_t2[k][:, j, :],
                                     start=False, stop=False)
                for j in range(NBLK):
                    nc.tensor.matmul(psum[:, j, :], wk, tap_t[k][:, j, :],
                                     start=False, stop=False)

            # ---- DVE stream: just-in-time diffs, taps k0..k8; I0 matmuls
            # are interleaved per-tap so the wd load isn't pulled early
            make_dx(-1)
            make_dx(0)
            make_dy(-1)
            make_dxy(-1)
            for k in (0, 1, 2):
                tap_products(k)
                mm_i0(k, start=(k == 0))
                mm_products(k)
            make_dx(1)
            make_dy(0)
            make_dxy(0)
            for k in (3, 4, 5):
                tap_products(k)
                mm_i0(k, start=False)
                mm_products(k)
            make_dx(2)
            make_dy(1)
            make_dxy(1)
            # I0 matmuls of the late taps go here: they only need A + wd,
            # and emitting them before the last products keeps them out of
            # the drain tail
            mm_i0(6, start=False)
            mm_i0(7, start=False)
            mm_i0(8, start=False)
            for k in (6, 7):
                tap_products(k)
                mm_products(k)
            tap_products(8, split_last=True)
            mm_products(8, stop=True)

            # ---- per-bank copy + store; copies split ACT/DVE (both idle by
            # now) with each engine's copies consecutive on its queue, and
            # stores split across the two HWDGE rings in bank order
            res = cp.tile([128, NBLK, W], f16, tag="res", name="res")
            outv = out.rearrange("(j p) c -> p j c", p=128)
            nc.scalar.activation(res[:, 0, :], psum[:, 0, :], ACTF.Copy)
            nc.sync.dma_start(out=outv[:, 0, :], in_=res[:, 0, :])
            nc.scalar.activation(res[:, 1, :], psum[:, 1, :], ACTF.Copy)
            nc.scalar.dma_start(out=outv[:, 1, :], in_=res[:, 1, :])
            nc.vector.tensor_copy(res[:, 2, :], psum[:, 2, :])
            nc.sync.dma_start(out=outv[:, 2, :], in_=res[:, 2, :])
            nc.vector.tensor_copy(res[:, 3, :], psum[:, 3, :])
            nc.scalar.dma_start(out=outv[:, 3, :], in_=res[:, 3, :])

    nc.compile()
    return nc


def kernel(input, weight, offset):
    global _compiled
    from concourse.bass_utils import run_bass_kernel_spmd

    if _compiled is None:
        _compiled = _build()
    nc = _compiled

    input = np.asarray(input, dtype=np.float32)
    offset = np.asarray(offset, dtype=np.float32)
    w9 = np.asarray(weight, dtype=np.float32).reshape(K * K)
    wdg = np.zeros((128, K * K, 128), np.float16)
    idx = np.arange(128)
    for k in range(K * K):
        wdg[idx, k, idx] = w9[k].astype(np.float16)

    ipad = np.zeros((B, HP, WP), np.float16)
    ipad[:, 1:H + 1, 1:W + 1] = input.astype(np.float16)

    # host-side cast + partition-major rearrange of the offsets:
    # (18, 512, 512) fp32 -> (18, 128, NBLK, 512) fp16 with row r = 128*j + p
    off16 = np.ascontiguousarray(
        offset.astype(np.float16)
        .reshape(B, 2 * K * K, NBLK, 128, W)
        .transpose(0, 1, 3, 2, 4))

    in_maps = [
        {"ipad": ipad[b], "off": off16[b], "wdg": wdg} for b in range(B)
    ]
    res = run_bass_kernel_spmd(nc, in_maps, list(range(NCORES)), trace=False)
    return np.stack(
        [res.results[b]["out"].astype(np.float32) for b in range(B)], axis=0)
